# revision 31
# baseline (speedup 1.0000x reference)
"""Trainium2 Bass kernel for a dense transformer block (pre-LN, FIRE attention
bias, GELU MLP), SPMD across 8 NeuronCores with zero collectives.

Sharding: core c handles batch b=c//2 with Q-token-tile parity par=c%2
(interleaved 128-row token tiles balance the causal-attention load). K/V are
recomputed locally for the full sequence; every sublayer is token-parallel.
Parity enters ONLY through input data (xq/xqb slices + mask packing), never
through addressing, so one graph runs on all 8 cores.

v2 (this file) vs the original baseline:
  * QKV / attn-proj / scores / AV run in fp8e4 with MatmulPerfMode.DoubleRow
    (2 packed K-rows per PE pass).  Weights are pre-scaled x32 on the host;
    epilogues fold 1/32 back.  Scores use a zero-padded DR pair (zeros chunk
    appended to ksb) so the 64-deep head contraction still gets DR rate.
  * The FIRE mask (packed x8 in fp8) is added to score PSUM by the PE itself
    via an identity-DR matmul (lhsT = [I | 0]), replacing the DVE tensor_add.
    exp then applies scale=1/8 and bias=-0.5 (softmax shift-invariant).
  * exp writes fp8 P tiles; AV and attn-proj consume fp8 directly.
  * y stays SBUF-resident (no DRAM ytd roundtrip); mask DMAs are batched per
    (j, 4-head group); weights ship in a handful of large DMAs.
  * rstd is computed as exp(-0.5*ln(var+eps)) so LN1/attention/LN2 share one
    ACT table (natural_log_exp) and the MLP's gelu is the only other load.
  * FC / MLP-proj stay bf16 (fp8 there fails the 2e-2 gate; weight-quant
    error dominates).  attention emission is interleaved with K/V/Q
    production and per-j attn-proj + LN2 so the PE never drains.
"""
import numpy as np
import ml_dtypes

import concourse.bass as bass
import concourse.bacc as bacc
import concourse.tile as tile
from concourse import mybir
from concourse.bass_utils import run_bass_kernel_spmd
from concourse.masks import make_identity

# Steer Bacc's act-table planner to one ln+exp table (and the gelu table):
# its greedy per-func choice otherwise alternates exp_and_others /
# natural_log and emits a LoadActFuncSet per LayerNorm rstd (~80us/iter).
# Positions are preserved (emptied sets) so act_func_set_id stays valid.
import concourse.bacc as _bacc_mod
from concourse.hw_specs import get_activation_tables as _gat
_KEEP_TABLES = {"natural_log_exp_and_others", "gelu_apprx_tanh_and_others"}


def _gat_filtered(arch):
    return {k: (v if k in _KEEP_TABLES else set())
            for k, v in _gat(arch).items()}


_bacc_mod.get_activation_tables = _gat_filtered

BF16NP = ml_dtypes.bfloat16
F32 = mybir.dt.float32
BF16 = mybir.dt.bfloat16
FP8 = mybir.dt.float8e4
F8NP = mybir.dt.np(FP8)
AF = mybir.ActivationFunctionType
DR = mybir.MatmulPerfMode.DoubleRow
ALU = mybir.AluOpType
P = 128
EPS = 1e-5
WS = 32.0          # host-side fp8 weight scale
WSI = 1.0 / WS
HG = 2             # heads per mask DMA group

FULL = dict(T=2048, C=1024, H=16, F=4096)
SMALL = dict(T=512, C=512, H=8, F=2048)


def cfg_derived(cfg):
    T, C, H, F = cfg["T"], cfg["C"], cfg["H"], cfg["F"]
    d = dict(cfg)
    d["HD"] = C // H
    d["NT"] = T // P
    d["NJ"] = T // P // 2
    d["NC"] = C // P
    d["NF"] = F // P
    d["HPW"] = P // d["HD"]          # heads per 128-wide wcol chunk
    d["C5"] = min(C, 512)
    d["NH5"] = C // d["C5"]
    d["T5"] = min(T, 512)
    d["NT5"] = T // d["T5"]
    d["NHG"] = H // HG
    return d


def mask_layout(d):
    """Flat element offsets of packed mask blocks, per (j, head-group)."""
    offs = {}
    off = 0
    for j in range(d["NJ"]):
        nkc = 2 * (j + 1)
        for g in range(d["NHG"]):
            offs[(j, g)] = off
            off += HG * nkc * P * P
    return offs, off


def build_graph(cfg, repeat=1, upto=99):
    d = cfg_derived(cfg)
    T, C, H, F, HD = d["T"], d["C"], d["H"], d["F"], d["HD"]
    NT, NJ, NC, NF, HPW = d["NT"], d["NJ"], d["NC"], d["NF"], d["HPW"]
    C5, NH5, T5, NT5 = d["C5"], d["NH5"], d["T5"], d["NT5"]
    NHG = d["NHG"]
    TQ = NJ * P
    moffs, MTOT = mask_layout(d)

    nc = bacc.Bacc("TRN2", target_bir_lowering=False, debug=False)

    xb = nc.dram_tensor("xb", [T, C], BF16, kind="ExternalInput").ap()
    xqb = nc.dram_tensor("xqb", [TQ, C], BF16, kind="ExternalInput").ap()
    xq = nc.dram_tensor("xq", [TQ, C], F32, kind="ExternalInput").ap()
    maskp = nc.dram_tensor("maskp", [MTOT], FP8, kind="ExternalInput").ap()
    wq_p = nc.dram_tensor("wq_p", [P, NC * NC * P], FP8, kind="ExternalInput").ap()
    wk_p = nc.dram_tensor("wk_p", [P, NC * NC * P], FP8, kind="ExternalInput").ap()
    wv_p = nc.dram_tensor("wv_p", [P, NC * C], FP8, kind="ExternalInput").ap()
    wap = nc.dram_tensor("wap", [P, NC * C], FP8, kind="ExternalInput").ap()
    wfc = nc.dram_tensor("wfc", [P, NF * NC * P], BF16, kind="ExternalInput").ap()
    wmp = nc.dram_tensor("wmp", [NF, P, C], BF16, kind="ExternalInput").ap()
    bq = nc.dram_tensor("bq", [C], F32, kind="ExternalInput").ap()
    bk = nc.dram_tensor("bk", [C], F32, kind="ExternalInput").ap()
    bvb = nc.dram_tensor("bvb", [C], BF16, kind="ExternalInput").ap()
    bapb = nc.dram_tensor("bapb", [C], BF16, kind="ExternalInput").ap()
    bmpb = nc.dram_tensor("bmpb", [C], BF16, kind="ExternalInput").ap()
    bfc = nc.dram_tensor("bfc", [F], F32, kind="ExternalInput").ap()
    out = nc.dram_tensor("out", [TQ, C], F32, kind="ExternalOutput").ap()

    with tile.TileContext(nc) as tc:
        with tc.tile_pool(name="consts", bufs=1) as consts:

            def bcast16(src1d, width, name):
                t = consts.tile([P, width], BF16, name=name)
                ap = bass.AP(tensor=src1d.tensor, offset=src1d.offset,
                             ap=[[0, P], [1, width]])
                nc.sync.dma_start(out=t, in_=ap)
                return t

            def colt(src1d, nchunks, name):
                t = consts.tile([P, nchunks], F32, name=name)
                ap = bass.AP(tensor=src1d.tensor, offset=src1d.offset,
                             ap=[[1, P], [P, nchunks]])
                nc.sync.dma_start(out=t, in_=ap)
                return t

            ident = consts.tile([P, P], BF16, name="ident")
            make_identity(nc, ident)
            identDR = consts.tile([P, 2, P], FP8, name="identDR")
            nc.gpsimd.memset(identDR, 0.0)
            make_identity(nc, identDR[:, 0, :], nomemset=True)
            eps_t = consts.tile([P, 1], F32, name="eps_t")
            nc.vector.memset(eps_t, EPS)
            zero_t = consts.tile([P, 1], F32, name="zero_t")
            nc.vector.memset(zero_t, 0.0)
            nb_t = consts.tile([P, 1], F32, name="nb_t")
            nc.vector.memset(nb_t, -0.5)
            bias_lhs = consts.tile([P, 2, P], FP8, name="bias_lhs")
            nc.gpsimd.memset(bias_lhs, 0.0)
            nc.gpsimd.memset(bias_lhs[0:1, 0, :], 1.0)
            bv8_b = consts.tile([P, C], FP8, name="bv8_b")
            ap8 = bass.AP(tensor=bvb.tensor, offset=bvb.offset,
                          ap=[[0, P], [1, C]])
            bv_b = bcast16(bvb, C, "bv_b")
            nc.gpsimd.tensor_scalar_mul(out=bv8_b, in0=bv_b, scalar1=WS)
            bap_b = bcast16(bapb, C, "bap_b")
            bmp_b = bcast16(bmpb, C, "bmp_b")
            bq_c = colt(bq, NC, "bq_c")
            bk_c = colt(bk, NC, "bk_c")
            bfc_c = colt(bfc, NF, "bfc_c")

            with tc.tile_pool(name="psA", bufs=3, space="PSUM") as psA, \
                 tc.tile_pool(name="psB", bufs=3, space="PSUM") as psB, \
                 tc.tile_pool(name="psC", bufs=2, space="PSUM") as psC, \
                 tc.tile_pool(name="w8p", bufs=1) as w8p, \
                 tc.tile_pool(name="mtp", bufs=3) as mtp, \
                 tc.tile_pool(name="ptp", bufs=4) as ptp, \
                 tc.tile_pool(name="ysm", bufs=4) as ysmp, \
                 tc.tile_pool(name="xqp", bufs=1) as xqp, \
                 tc.tile_pool(name="lnh", bufs=2) as lnh, \
                 tc.tile_pool(name="lns", bufs=3) as lns, \
                 tc.tile_pool(name="lnx", bufs=2) as lnx:
                for rep in range(repeat):
                    _emit_iteration(upto,
                        nc, tc, d, rep,
                        ident=ident, identDR=identDR, eps_t=eps_t,
                        zero_t=zero_t, nb_t=nb_t, bv8_b=bv8_b,
                        bias_lhs=bias_lhs, bap_b=bap_b,
                        bmp_b=bmp_b, bq_c=bq_c, bk_c=bk_c, bfc_c=bfc_c,
                        xb=xb, xqb=xqb, xq=xq, maskp=maskp, moffs=moffs,
                        wq_p=wq_p, wk_p=wk_p, wv_p=wv_p, wap=wap, wfc=wfc,
                        wmp=wmp, out=out,
                        psA=psA, psB=psB, psC=psC, w8p=w8p, mtp=mtp,
                        ptp=ptp, ysmp=ysmp, xqp=xqp, lnh=lnh, lns=lns,
                        lnx=lnx)
    nc.compile()
    return nc


def _emit_iteration(upto, nc, tc, d, rep, *, ident, identDR, eps_t, zero_t,
                    nb_t, bv8_b, bias_lhs, bap_b, bmp_b, bq_c, bk_c, bfc_c,
                    xb, xqb, xq, maskp, moffs, wq_p, wk_p, wv_p, wap, wfc,
                    wmp, out, psA, psB, psC, w8p, mtp, ptp, ysmp, xqp, lnh, lns, lnx):
    T, C, H, F, HD = d["T"], d["C"], d["H"], d["F"], d["HD"]
    NT, NJ, NC, NF, HPW = d["NT"], d["NJ"], d["NC"], d["NF"], d["HPW"]
    C5, NH5, T5, NT5 = d["C5"], d["NH5"], d["T5"], d["NT5"]
    NHG = d["NHG"]
    TQ = NJ * P
    NP2 = NC // 2                    # DR contraction pairs over C
    sfx = f"_r{rep}"

    # -------- persistent activations (alloc order = reverse free order) -----
    x2sb, free_x2 = tc.tile([P, NJ, C], F32, name="x2sb" + sfx)
    h2T, free_h2T = tc.tile([P, NC, TQ], BF16, name="h2T" + sfx)
    vaug, free_v = tc.tile([P, NT, H, HD + 1], FP8, name="vaug" + sfx)
    qsb, free_q = tc.tile([P, NC, TQ], FP8, name="qsb" + sfx)
    ksb, free_k = tc.tile([P, NC + 1, T], FP8, name="ksb" + sfx)
    ysb, free_y = tc.tile([P, NC, TQ], FP8, name="ysb" + sfx)
    hqT, free_hqT = tc.tile([P, NC, TQ], FP8, name="hqT" + sfx)
    h1T, free_h1T = tc.tile([P, NC, T], FP8, name="h1T" + sfx)
    nc.vector.memset(vaug[:, :, :, HD:HD + 1], WS)
    nc.gpsimd.memset(ksb[:, NC, :], 0.0)    # zeros chunk for score-DR padding

    # -------- resident fp8 weights (few big DMAs, issued by first use) -----
    wqt = w8p.tile([P, NC, NC, P], FP8, name="wqt")
    wkt = w8p.tile([P, NC, NC, P], FP8, name="wkt")
    wvt = w8p.tile([P, NC, C], FP8, name="wvt")
    wapt = w8p.tile([P, NC, C], FP8, name="wapt")

    def emit_w_kv():
        nc.sync.dma_start(out=wkt,
                          in_=wk_p.rearrange("p (mc ci q) -> p mc ci q",
                                             ci=NC, q=P))
        nc.sync.dma_start(out=wvt,
                          in_=wv_p.rearrange("p (ci q) -> p ci q", q=C))

    def emit_w_qp():
        nc.sync.dma_start(out=wqt,
                          in_=wq_p.rearrange("p (mc ci q) -> p mc ci q",
                                             ci=NC, q=P))
        nc.sync.dma_start(out=wapt,
                          in_=wap.rearrange("p (ci q) -> p ci q", q=C))

    # ---------------- LayerNorm helpers ----------------
    def ln_stats(xt):
        """[128, C] -> rstd [P,1], nmu = -mu*rstd (exp(-.5*ln(var+eps)))."""
        ns = max(1, C // 512)
        w = C // ns
        st = lns.tile([P, ns * 6 + 5], F32, name="st")
        stats = st[:, 0:ns * 6].rearrange("p (s x) -> p s x", x=6)
        for s in range(ns):
            nc.vector.bn_stats(out=stats[:, s, :], in_=xt[:, s * w:(s + 1) * w])
        mv = st[:, ns * 6:ns * 6 + 2]
        nc.vector.bn_aggr(out=mv, in_=stats)
        lv = st[:, ns * 6 + 2:ns * 6 + 3]
        nc.scalar.activation(out=lv, in_=mv[:, 1:2], func=AF.Ln,
                             bias=eps_t, scale=1.0)
        rstd = st[:, ns * 6 + 3:ns * 6 + 4]
        nc.scalar.activation(out=rstd, in_=lv, func=AF.Exp,
                             bias=zero_t, scale=-0.5)
        nmu = st[:, ns * 6 + 4:ns * 6 + 5]
        nc.vector.tensor_mul(out=nmu, in0=mv[:, 0:1], in1=rstd)
        nc.vector.tensor_scalar_mul(out=nmu, in0=nmu, scalar1=-1.0)
        return rstd, nmu

    def ln_tile(xt, alt, all_dve=False):
        rstd, nmu = ln_stats(xt)
        hb = lnh.tile([P, C], BF16, name="hb")
        nc.gpsimd.tensor_scalar(out=hb, in0=xt, scalar1=rstd,
                                scalar2=nmu, op0=ALU.mult, op1=ALU.add)
        return hb

    def ln_transposed(hb, dst, idx, use_act=False):
        """transpose [128,C] bf16 into dst chunks (LN gain/bias are folded
        into the consuming weights host-side, so these are plain copies,
        fused 4 chunks per op; early chunks go to the then-idle ACT)."""
        for c0 in range(0, NC, 4):
            pt4 = psB.tile([P, 4, P], BF16, name="sps")
            for i in range(4):
                nc.tensor.transpose(pt4[:, i, :],
                                    hb[:, (c0 + i) * P:(c0 + i + 1) * P], ident)
            if use_act:
                nc.scalar.copy(
                    out=dst[:, c0:c0 + 4, idx * P:(idx + 1) * P], in_=pt4)
            else:
                nc.vector.tensor_copy(
                    out=dst[:, c0:c0 + 4, idx * P:(idx + 1) * P], in_=pt4)

    # ---------------- LN1 helpers (emitted inside the K-chunk loop) -------
    lnp = lnx
    GT = 2                           # token tiles per batched load

    def bq4(wq0):
        b = bq_c[:, wq0:wq0 + 4]
        return bass.AP(tensor=b.tensor, offset=b.offset,
                       ap=[b.ap[0], [1, 4], [0, P]])

    def emit_ln1_pair(g0):
        xts = lnp.tile([P, GT, C], BF16, name="xts")
        nc.sync.dma_start(
            out=xts, in_=xb[g0 * P:(g0 + GT) * P, :]
            .rearrange("(t p) c -> p t c", p=P))
        for tl in range(GT):
            hb = ln_tile(xts[:, tl, :], (g0 + tl) % 2 == 0)
            ln_transposed(hb, h1T, g0 + tl, use_act=(g0 < 8))

    def emit_ln1_xq(j):
        xts = lnp.tile([P, GT, C], BF16, name="xts")
        nc.sync.dma_start(out=xts[:, 0, :], in_=xqb[j * P:(j + 1) * P, :])
        hb = ln_tile(xts[:, 0, :], j % 2 == 0)
        ln_transposed(hb, hqT, j, use_act=(j < 4))

    # ---------------- fused LN1 + K/V/Q + attention + proj + LN2 ----------
    DRN = 256                       # max N per DR matmul (moving = 2*DRN)

    def dr_accum(ps, n0, nn, lhs_of, rhs_of):
        """accumulate over NP2 chunk-pairs into ps[:, n0:n0+nn]."""
        for sub0 in range(0, nn, DRN):
            sw = min(DRN, nn - sub0)
            for cp in range(NP2):
                nc.tensor.matmul(
                    ps[:, n0 + sub0:n0 + sub0 + sw],
                    lhsT=lhs_of(cp), rhs=rhs_of(cp, n0 + sub0, sw),
                    start=(cp == 0), stop=(cp == NP2 - 1), perf_mode=DR)

    def emit_k1(tt, wk):
        ps = psA.tile([P, 512], F32, name="ps")[:, :T5]
        dr_accum(ps, 0, T5,
                 lambda cp: wkt[:, wk, 2 * cp:2 * cp + 2, :],
                 lambda cp, o, w: h1T[:, 2 * cp:2 * cp + 2,
                                      tt * T5 + o:tt * T5 + o + w])
        if tt < 2 or wk % 2 == 0:
            nc.scalar.activation(
                out=ksb[:, wk, tt * T5:(tt + 1) * T5], in_=ps,
                func=AF.Identity, bias=bk_c[:, wk:wk + 1], scale=WSI)
        else:
            nc.vector.tensor_scalar(
                out=ksb[:, wk, tt * T5:(tt + 1) * T5], in0=ps,
                scalar1=WSI, scalar2=bk_c[:, wk:wk + 1],
                op0=ALU.mult, op1=ALU.add)

    hpv = C5 // HD

    def emit_v(t):
        for vh in range(NH5):
            ps = psA.tile([P, 512], F32, name="ps")[:, :C5]
            for sub0 in range(0, C5, DRN):
                sw = min(DRN, C5 - sub0)
                for cp in range(NP2):
                    nc.tensor.matmul(
                        ps[:, sub0:sub0 + sw],
                        lhsT=h1T[:, 2 * cp:2 * cp + 2, t * P:(t + 1) * P],
                        rhs=wvt[:, 2 * cp:2 * cp + 2,
                                vh * C5 + sub0:vh * C5 + sub0 + sw],
                        start=(cp == 0), stop=False, perf_mode=DR)
                nc.tensor.matmul(
                    ps[:, sub0:sub0 + sw], lhsT=bias_lhs,
                    rhs=dup2(bv8_b[:, vh * C5 + sub0:vh * C5 + sub0 + sw], sw),
                    start=False, stop=True, perf_mode=DR)
            dst = vaug[:, t, vh * hpv:(vh + 1) * hpv, 0:HD]
            srcv = ps.rearrange("p (h dd) -> p h dd", dd=HD)
            if t < 8:
                nc.scalar.copy(out=dst, in_=srcv)
            else:
                nc.vector.tensor_copy(out=dst, in_=srcv)

    def emit_q4(j, wq0):
        ps4 = psB.tile([P, 4, P], F32, name="sps")
        for i in range(4):
            dr_accum(ps4[:, i, :], 0, P,
                     lambda cp, wq=wq0 + i: wqt[:, wq, 2 * cp:2 * cp + 2, :],
                     lambda cp, o, w: hqT[:, 2 * cp:2 * cp + 2,
                                          j * P + o:j * P + o + w])
        # bq varies per wq chunk -> per-(partition,chunk) bias tile view
        nc.vector.scalar_tensor_tensor(
            out=qsb[:, wq0:wq0 + 4, j * P:(j + 1) * P], in0=ps4,
            scalar=WSI, in1=bq4(wq0), op0=ALU.mult, op1=ALU.add)

    # ------- attention -------
    _sctr = [0]

    def dup2(apx, n):
        """[p, n] -> [p, 2, n] with stride-0 duplicated middle dim."""
        return bass.AP(tensor=apx.tensor, offset=apx.offset,
                       ap=[apx.ap[0], [0, 2], [1, n]])

    def score_group(j, h, kc0, kw, mj, nkc):
        hp = (h % HPW) * HD
        wk = h // HPW
        hl = h % HG
        _sctr[0] += 1
        pool = psA if _sctr[0] % 2 else psB
        sps = pool.tile([P, 4, P], F32, name="ps" if _sctr[0] % 2 else "sps")
        for ki in range(kw):
            kc = kc0 + ki
            kb = ksb[hp:hp + HD, wk, kc * P:(kc + 1) * P]
            lhsT = bass.AP(tensor=kb.tensor, offset=kb.offset,
                           ap=[kb.ap[0], [(NC - wk) * T, 2], [1, P]])
            qb = qsb[hp:hp + HD, wk, j * P:(j + 1) * P]
            nc.tensor.matmul(sps[:, ki, :], lhsT=lhsT, rhs=dup2(qb, P),
                             start=True, stop=False, perf_mode=DR)
            mb = mj[:, (hl * nkc + kc) * P:(hl * nkc + kc + 1) * P]
            nc.tensor.matmul(sps[:, ki, :], lhsT=identDR, rhs=dup2(mb, P),
                             start=False, stop=True, perf_mode=DR)
        pt = ptp.tile([P, 4, P], FP8, name="pt")
        nc.scalar.activation(out=pt[:, :kw, :], in_=sps[:, :kw, :],
                             func=AF.Exp, bias=nb_t, scale=0.125)
        return pt

    def av_group(h, kc0, kw, pt, yps, nkc):
        for kp in range(0, kw, 2):
            kc = kc0 + kp
            nc.tensor.matmul(
                yps, lhsT=pt[:, kp:kp + 2, :],
                rhs=vaug[:, kc:kc + 2, h, :],
                start=(kc == 0), stop=(kc + 2 >= nkc), perf_mode=DR)

    def finish_pair(j, hA, yps2):
        wk = hA // 2
        rec = ysmp.tile([P, 2], F32, name="rec")
        nc.vector.reciprocal(out=rec, in_=yps2[:, :, HD])
        ynm = ysmp.tile([P, P], BF16, name="ynm")
        nc.vector.tensor_scalar_mul(out=ynm[:, 0:HD],
                                    in0=yps2[:, 0, 0:HD],
                                    scalar1=rec[:, 0:1])
        nc.vector.tensor_scalar_mul(out=ynm[:, HD:2 * HD],
                                    in0=yps2[:, 1, 0:HD],
                                    scalar1=rec[:, 1:2])
        ypt = psB.tile([P, P], BF16, name="sps")
        nc.tensor.transpose(ypt, ynm, ident)
        nc.vector.tensor_copy(out=ysb[:, wk, j * P:(j + 1) * P], in_=ypt)

    _pend = [None]     # AV lag-1 pipeline state, global across pairs and j

    def flush_av():
        p = _pend[0]
        if p is None:
            return
        j, hA, kc0, kw, ptA, ptB, yps2, nkc, last = p
        av_group(hA, kc0, kw, ptA, yps2[:, 0, :], nkc)
        av_group(hA + 1, kc0, kw, ptB, yps2[:, 1, :], nkc)
        if last:
            finish_pair(j, hA, yps2)
        _pend[0] = None

    bg = []                        # (need_by_j, emit_fn) background quanta

    def bg_pop(need_j):
        i = 0
        while i < len(bg):
            if bg[i][0] <= need_j:
                bg.pop(i)[1]()
            else:
                i += 1

    def bg_pop_one():
        n = 2 if len(bg) > 6 else 1
        for _ in range(min(n, len(bg))):
            bg.pop(0)[1]()

    _mjs = {}

    def emit_mask(j, g):
        nkc = 2 * (j + 1)
        mj = mtp.tile([P, HG * 2 * NJ * P], FP8, name="mj")
        msrc = bass.AP(tensor=maskp.tensor, offset=moffs[(j, g)],
                       ap=[[HG * nkc * P, P], [1, HG * nkc * P]])
        nc.sync.dma_start(out=mj[:, :HG * nkc * P], in_=msrc)
        _mjs.setdefault(j, []).append(mj)

    _mstream = [(j_, g_) for j_ in range(NJ) for g_ in range(NHG)]
    _mptr = [0]

    def mask_ensure(upto):
        while _mptr[0] <= min(upto, len(_mstream) - 1):
            j_, g_ = _mstream[_mptr[0]]
            emit_mask(j_, g_)
            _mptr[0] += 1

    def emit_attention(j):
        nkc = 2 * (j + 1)
        mask_ensure(j * NHG + 1)
        bg_pop(j)                   # anything attention(j) depends on
        mjs = _mjs[j]
        for hA in range(0, H, 2):
            hB = hA + 1
            mask_ensure(j * NHG + hA // HG + 2)
            yps2 = psC.tile([P, 2, HD + 1], F32, name="yps")
            for kc0 in range(0, nkc, 4):
                kw = min(4, nkc - kc0)
                ptA = score_group(j, hA, kc0, kw, mjs[hA // HG], nkc)
                ptB = score_group(j, hB, kc0, kw, mjs[hB // HG], nkc)
                flush_av()
                _pend[0] = (j, hA, kc0, kw, ptA, ptB, yps2, nkc,
                            kc0 + kw >= nkc)
                bg_pop_one()

    def emit_proj_ln2(j):
        xqt = xqp.tile([P, C], F32, name="xqt")
        nc.sync.dma_start(out=xqt, in_=xq[j * P:(j + 1) * P, :])
        pss = []
        for nh in range(NH5):
            ps = psA.tile([P, 512], F32, name="ps")[:, :C5]
            dr_accum(ps, 0, C5,
                     lambda cp: ysb[:, 2 * cp:2 * cp + 2, j * P:(j + 1) * P],
                     lambda cp, o, w: wapt[:, 2 * cp:2 * cp + 2,
                                           nh * C5 + o:nh * C5 + o + w])
            pss.append(ps)
        for nh in range(NH5):
            sl = slice(nh * C5, (nh + 1) * C5)
            nc.vector.scalar_tensor_tensor(
                out=x2sb[:, j, sl], in0=pss[nh], scalar=WSI,
                in1=xqt[:, sl], op0=ALU.mult, op1=ALU.add)
        nc.gpsimd.tensor_add(out=x2sb[:, j, :], in0=x2sb[:, j, :], in1=bap_b)
        hb = ln_tile(x2sb[:, j, :], j % 2 == 0, all_dve=True)
        ln_transposed(hb, h2T, j)

    TPC = T5 // P                  # token tiles per K-chunk

    def bg_for_chunk(tt):
        need = 2 * tt
        q = []
        for g0 in range(tt * TPC, (tt + 1) * TPC, GT):
            q.append((need, (lambda g=g0: emit_ln1_pair(g))))
        for wk in range(0, NC, 2):
            q.append((need, (lambda w=wk: (emit_k1(tt, w),
                                           emit_k1(tt, w + 1)))))
        for t in range(tt * TPC, (tt + 1) * TPC, 2):
            q.append((need, (lambda t_=t: (emit_v(t_), emit_v(t_ + 1)))))
        return q

    def bg_for_j(j):
        q = [(j, lambda: emit_ln1_xq(j))]
        for wq0 in range(0, NC, 4):
            q.append((j, (lambda w=wq0: emit_q4(j, w))))
        return q

    # prologue: chunk 0 + j0 prerequisites, DMAs ordered by first use
    pro = bg_for_chunk(0)
    nln = max(1, TPC // GT)
    for _, f in pro[:nln]:
        f()
    emit_w_kv()
    for _, f in pro[nln:]:
        f()
    emit_w_qp()
    for _, f in bg_for_j(0):
        f()
    if NJ > 1:
        for _, f in bg_for_j(1):
            f()
    for j in range(NJ):
        if j % 2 == 0 and j // 2 + 1 < NT5:
            bg.extend(bg_for_chunk(j // 2 + 1))
        if j + 2 < NJ:
            bg.extend(bg_for_j(j + 2))
        emit_attention(j)
        if j > 0:
            emit_proj_ln2(j - 1)
    bg_pop(NJ)
    flush_av()
    emit_proj_ln2(NJ - 1)
    free_h1T()
    free_hqT()
    free_y()
    free_k()
    free_q()
    free_v()
    if upto <= 2:
        free_h2T(); free_x2()
        return

    # mlp-proj bias folded into the x2 accumulator up front
    for j in range(NJ):
        nc.gpsimd.tensor_add(out=x2sb[:, j, :], in0=x2sb[:, j, :], in1=bmp_b)

    # ---------------- FC+GELU -> aT, then MLP-proj accumulated into x2 -----
    GRP = 4
    NG = NF // GRP
    aT, free_aT = tc.tile([P, NF, TQ], BF16, name="aT" + sfx)
    with tc.tile_pool(name="wfcp" + sfx, bufs=2) as wfcp, \
         tc.tile_pool(name="wmpp" + sfx, bufs=1) as wmpp:
        for g in range(NG):
            wfg = wfcp.tile([P, GRP, NC, P], BF16, name="wfg")
            nc.sync.dma_start(
                out=wfg,
                in_=bass.AP(tensor=wfc.tensor,
                            offset=wfc.offset + g * GRP * NC * P,
                            ap=[[NF * NC * P, P], [1, GRP * NC * P]])
                .rearrange("p (wf ci q) -> p wf ci q", ci=NC, q=P))
            for wfl in range(GRP):
                wf = g * GRP + wfl
                for tq5 in range(TQ // 512 if TQ >= 512 else 1):
                    Q5 = min(512, TQ)
                    ps = psA.tile([P, 512], F32, name="ps")[:, :Q5]
                    for ci in range(NC):
                        nc.tensor.matmul(
                            ps, lhsT=wfg[:, wfl, ci, :],
                            rhs=h2T[:, ci, tq5 * Q5:(tq5 + 1) * Q5],
                            start=(ci == 0), stop=(ci == NC - 1))
                    nc.scalar.activation(
                        out=aT[:, wf, tq5 * Q5:(tq5 + 1) * Q5], in_=ps,
                        func=AF.Gelu_apprx_tanh, bias=bfc_c[:, wf:wf + 1],
                        scale=1.0)
            wmg = wmpp.tile([P, GRP, C], BF16, name="wmg")
            nc.sync.dma_start(
                out=wmg,
                in_=wmp[g * GRP:(g + 1) * GRP, :, :].rearrange(
                    "fi p q -> p fi q"))
            for j in range(NJ):
                pss = []
                for nh in range(NH5):
                    pss.append(psB.tile([P, 512], F32, name="sps")[:, :C5])
                for fi in range(GRP):
                    for nh in range(NH5):
                        nc.tensor.matmul(
                            pss[nh],
                            lhsT=aT[:, g * GRP + fi, j * P:(j + 1) * P],
                            rhs=wmg[:, fi, nh * C5:(nh + 1) * C5],
                            start=(fi == 0), stop=(fi == GRP - 1))
                for nh in range(NH5):
                    sl = slice(nh * C5, (nh + 1) * C5)
                    nc.vector.tensor_add(out=x2sb[:, j, sl],
                                         in0=x2sb[:, j, sl], in1=pss[nh])
    free_aT()
    free_h2T()

    # ---------------- write out ----------------
    for j in range(NJ):
        nc.sync.dma_start(out=out[j * P:(j + 1) * P, :], in_=x2sb[:, j, :])
    free_x2()


# ======================= host side =======================

def prep_shards(inputs, cfg):
    d = cfg_derived(cfg)
    T, C, H, F, HD = d["T"], d["C"], d["H"], d["F"], d["HD"]
    NJ, NC, NF, NHG = d["NJ"], d["NC"], d["NF"], d["NHG"]
    moffs, MTOT = mask_layout(d)

    x = np.ascontiguousarray(np.asarray(inputs["x"], np.float32))
    mask = np.asarray(inputs["fire_causal_mask"], np.float32)[0]  # [H,T,T]
    wqkv = np.asarray(inputs["w_qkv"], np.float32)
    bqkv = np.asarray(inputs["b_qkv"], np.float32)
    # fold LN gains/biases into the consuming weights (device LN emits the
    # pre-gain normalized activations only)
    g1 = np.asarray(inputs["ln1_g"], np.float32)
    b1 = np.asarray(inputs["ln1_b"], np.float32)
    g2 = np.asarray(inputs["ln2_g"], np.float32)
    b2 = np.asarray(inputs["ln2_b"], np.float32)
    wqkv_g = wqkv * g1[:, None]
    bqkv_f = bqkv + b1 @ wqkv
    wfc_raw = np.asarray(inputs["w_fc"], np.float32)
    wfc_g = wfc_raw * g2[:, None]
    bfc_f = np.asarray(inputs["b_fc"], np.float32) + b2 @ wfc_raw

    def q8(w):
        return np.clip(w * WS, -240., 240.).astype(F8NP)

    def tile_kxm_f8(w):
        """[K, M] -> [P, M//P, K//P, P] flat: row p holds w[ci*P+p, mc*P+q]
        packed [mc][ci][q] — the DR lhsT layout, one contiguous DMA."""
        Kd, M = w.shape
        w4 = q8(w).reshape(Kd // P, P, M // P, P)       # [ci, p, mc, q]
        t = w4.transpose(1, 2, 0, 3).reshape(P, -1)     # [p, mc, ci, q]
        return np.ascontiguousarray(t)

    def tile_rhs_f8(w):
        """[K, N] -> [P, K//P * N]: row p holds w[ci*P+p, :] ci-major."""
        Kd, N = w.shape
        t = q8(w).reshape(Kd // P, P, N).transpose(1, 0, 2).reshape(P, -1)
        return np.ascontiguousarray(t)

    def tile_kxm_bf(w):
        Kd, M = w.shape
        w4 = w.astype(BF16NP).reshape(Kd // P, P, M // P, P)
        t = w4.transpose(1, 2, 0, 3).reshape(P, -1)
        return np.ascontiguousarray(t)

    # mask: packed 8*m fp8, per (j, head-group) contiguous [p(kt), h, kc, q]
    m8 = np.clip(mask * 8.0, -240., 240.).astype(F8NP)
    maskps = []
    for par in range(2):
        buf = np.empty(MTOT, F8NP)
        for j in range(NJ):
            nkc = 2 * (j + 1)
            tq = 2 * j + par
            sub = m8[:, tq * P:(tq + 1) * P, :nkc * P]      # [H, q, nkc*P]
            arr = sub.reshape(H, P, nkc, P).transpose(3, 0, 2, 1)
            for g in range(NHG):
                o = moffs[(j, g)]
                blk = arr[:, g * HG:(g + 1) * HG]           # [p, HG, kc, q]
                buf[o:o + blk.size] = blk.ravel()
        maskps.append(buf)

    shared = dict(
        wq_p=tile_kxm_f8(wqkv_g[:, :C]),
        wk_p=tile_kxm_f8(wqkv_g[:, C:2 * C]),
        wv_p=tile_rhs_f8(wqkv_g[:, 2 * C:]),
        wap=tile_rhs_f8(np.asarray(inputs["w_attn_proj"], np.float32)),
        wfc=tile_kxm_bf(wfc_g),
        wmp=np.ascontiguousarray(
            np.asarray(inputs["w_mlp_proj"], np.float32)
            .reshape(NF, P, C).astype(BF16NP)),
        bq=bqkv_f[:C].copy(), bk=bqkv_f[C:2 * C].copy(),
        bvb=bqkv_f[2 * C:].astype(BF16NP),
        bapb=np.asarray(inputs["b_attn_proj"], np.float32).astype(BF16NP),
        bmpb=np.asarray(inputs["b_mlp_proj"], np.float32).astype(BF16NP),
        bfc=bfc_f,
    )
    in_maps = []
    for c in range(8):
        b, par = c // 2, c % 2
        xq_ = np.concatenate(
            [x[b, (2 * j + par) * P:(2 * j + par + 1) * P] for j in range(NJ)], 0)
        m = dict(shared)
        m["xb"] = x[b].astype(BF16NP)
        m["xq"] = np.ascontiguousarray(xq_)
        m["xqb"] = np.ascontiguousarray(xq_).astype(BF16NP)
        m["maskp"] = maskps[par]
        in_maps.append(m)
    return in_maps


def assemble(results, cfg, B=4):
    d = cfg_derived(cfg)
    T, C, NJ = d["T"], d["C"], d["NJ"]
    out = np.zeros((B, T, C), np.float32)
    for c in range(8):
        b, par = c // 2, c % 2
        co = results[c]["out"]
        for j in range(NJ):
            tq = 2 * j + par
            out[b, tq * P:(tq + 1) * P] = co[j * P:(j + 1) * P]
    return out


_GRAPH_CACHE = {}


def kernel(**inputs):
    cfg = FULL
    key = "full"
    if key not in _GRAPH_CACHE:
        _GRAPH_CACHE[key] = build_graph(cfg)
    nc = _GRAPH_CACHE[key]
    in_maps = prep_shards(inputs, cfg)
    res = run_bass_kernel_spmd(nc, in_maps, core_ids=list(range(8)))
    return assemble(res.results, cfg)


# revision 33
# speedup vs baseline: 1.0229x; 1.0229x over previous
"""Trainium2 Bass kernel for a dense transformer block (pre-LN, FIRE attention
bias, GELU MLP), SPMD across 8 NeuronCores with zero collectives.

Sharding: core c handles batch b=c//2 with Q-token-tile parity par=c%2
(interleaved 128-row token tiles balance the causal-attention load). K/V are
recomputed locally for the full sequence; every sublayer is token-parallel.
Parity enters ONLY through input data (xq/xqb slices + mask packing), never
through addressing, so one graph runs on all 8 cores.

v2 (this file) vs the original baseline:
  * QKV / attn-proj / scores / AV run in fp8e4 with MatmulPerfMode.DoubleRow
    (2 packed K-rows per PE pass).  Weights are pre-scaled x32 on the host;
    epilogues fold 1/32 back.  Scores use a zero-padded DR pair (zeros chunk
    appended to ksb) so the 64-deep head contraction still gets DR rate.
  * The FIRE mask (packed x8 in fp8) is added to score PSUM by the PE itself
    via an identity-DR matmul (lhsT = [I | 0]), replacing the DVE tensor_add.
    exp then applies scale=1/8 and bias=-0.5 (softmax shift-invariant).
  * exp writes fp8 P tiles; AV and attn-proj consume fp8 directly.
  * y stays SBUF-resident (no DRAM ytd roundtrip); mask DMAs are batched per
    (j, 4-head group); weights ship in a handful of large DMAs.
  * rstd is computed as exp(-0.5*ln(var+eps)) so LN1/attention/LN2 share one
    ACT table (natural_log_exp) and the MLP's gelu is the only other load.
  * FC / MLP-proj stay bf16 (fp8 there fails the 2e-2 gate; weight-quant
    error dominates).  attention emission is interleaved with K/V/Q
    production and per-j attn-proj + LN2 so the PE never drains.
"""
import numpy as np
import ml_dtypes

import concourse.bass as bass
import concourse.bacc as bacc
import concourse.tile as tile
from concourse import mybir
from concourse.bass_utils import run_bass_kernel_spmd
from concourse.masks import make_identity

# Steer Bacc's act-table planner to one ln+exp table (and the gelu table):
# its greedy per-func choice otherwise alternates exp_and_others /
# natural_log and emits a LoadActFuncSet per LayerNorm rstd (~80us/iter).
# Positions are preserved (emptied sets) so act_func_set_id stays valid.
import concourse.bacc as _bacc_mod
from concourse.hw_specs import get_activation_tables as _gat
_KEEP_TABLES = {"natural_log_exp_and_others", "gelu_apprx_tanh_and_others"}


def _gat_filtered(arch):
    return {k: (v if k in _KEEP_TABLES else set())
            for k, v in _gat(arch).items()}


_bacc_mod.get_activation_tables = _gat_filtered

BF16NP = ml_dtypes.bfloat16
F32 = mybir.dt.float32
BF16 = mybir.dt.bfloat16
FP8 = mybir.dt.float8e4
F8NP = mybir.dt.np(FP8)
AF = mybir.ActivationFunctionType
DR = mybir.MatmulPerfMode.DoubleRow
ALU = mybir.AluOpType
P = 128
EPS = 1e-5
WS = 32.0          # host-side fp8 weight scale
WSI = 1.0 / WS
HG = 2             # heads per mask DMA group

FULL = dict(T=2048, C=1024, H=16, F=4096)
SMALL = dict(T=512, C=512, H=8, F=2048)


def cfg_derived(cfg):
    T, C, H, F = cfg["T"], cfg["C"], cfg["H"], cfg["F"]
    d = dict(cfg)
    d["HD"] = C // H
    d["NT"] = T // P
    d["NJ"] = T // P // 2
    d["NC"] = C // P
    d["NF"] = F // P
    d["HPW"] = P // d["HD"]          # heads per 128-wide wcol chunk
    d["C5"] = min(C, 512)
    d["NH5"] = C // d["C5"]
    d["T5"] = min(T, 512)
    d["NT5"] = T // d["T5"]
    d["NHG"] = H // HG
    return d


def mask_layout(d):
    """Flat element offsets of packed mask blocks, per (j, head-group)."""
    offs = {}
    off = 0
    for j in range(d["NJ"]):
        nkc = 2 * (j + 1)
        for g in range(d["NHG"]):
            offs[(j, g)] = off
            off += HG * nkc * P * P
    return offs, off


def build_graph(cfg, repeat=1, upto=99):
    d = cfg_derived(cfg)
    T, C, H, F, HD = d["T"], d["C"], d["H"], d["F"], d["HD"]
    NT, NJ, NC, NF, HPW = d["NT"], d["NJ"], d["NC"], d["NF"], d["HPW"]
    C5, NH5, T5, NT5 = d["C5"], d["NH5"], d["T5"], d["NT5"]
    NHG = d["NHG"]
    TQ = NJ * P
    moffs, MTOT = mask_layout(d)

    nc = bacc.Bacc("TRN2", target_bir_lowering=False, debug=False)

    xb = nc.dram_tensor("xb", [T, C], BF16, kind="ExternalInput").ap()
    xqb = nc.dram_tensor("xqb", [TQ, C], BF16, kind="ExternalInput").ap()
    xq = nc.dram_tensor("xq", [TQ, C], F32, kind="ExternalInput").ap()
    maskp = nc.dram_tensor("maskp", [MTOT], FP8, kind="ExternalInput").ap()
    wq_p = nc.dram_tensor("wq_p", [P, NC * NC * P], FP8, kind="ExternalInput").ap()
    wk_p = nc.dram_tensor("wk_p", [P, NC * NC * P], FP8, kind="ExternalInput").ap()
    wv_p = nc.dram_tensor("wv_p", [P, NC * C], FP8, kind="ExternalInput").ap()
    wap = nc.dram_tensor("wap", [P, NC * C], FP8, kind="ExternalInput").ap()
    wfc = nc.dram_tensor("wfc", [P, NF * NC * P], BF16, kind="ExternalInput").ap()
    wmp = nc.dram_tensor("wmp", [NF, P, C], BF16, kind="ExternalInput").ap()
    bq = nc.dram_tensor("bq", [C], F32, kind="ExternalInput").ap()
    bk = nc.dram_tensor("bk", [C], F32, kind="ExternalInput").ap()
    bvb = nc.dram_tensor("bvb", [C], BF16, kind="ExternalInput").ap()
    bapb = nc.dram_tensor("bapb", [C], BF16, kind="ExternalInput").ap()
    bmpb = nc.dram_tensor("bmpb", [C], BF16, kind="ExternalInput").ap()
    bfc = nc.dram_tensor("bfc", [F], F32, kind="ExternalInput").ap()
    out = nc.dram_tensor("out", [TQ, C], F32, kind="ExternalOutput").ap()

    with tile.TileContext(nc) as tc:
        with tc.tile_pool(name="consts", bufs=1) as consts:

            def bcast16(src1d, width, name):
                t = consts.tile([P, width], BF16, name=name)
                ap = bass.AP(tensor=src1d.tensor, offset=src1d.offset,
                             ap=[[0, P], [1, width]])
                nc.sync.dma_start(out=t, in_=ap)
                return t

            def colt(src1d, nchunks, name):
                t = consts.tile([P, nchunks], F32, name=name)
                ap = bass.AP(tensor=src1d.tensor, offset=src1d.offset,
                             ap=[[1, P], [P, nchunks]])
                nc.sync.dma_start(out=t, in_=ap)
                return t

            ident = consts.tile([P, P], BF16, name="ident")
            make_identity(nc, ident)
            identDR = consts.tile([P, 2, P], FP8, name="identDR")
            nc.gpsimd.memset(identDR, 0.0)
            make_identity(nc, identDR[:, 0, :], nomemset=True)
            eps_t = consts.tile([P, 1], F32, name="eps_t")
            nc.vector.memset(eps_t, EPS)
            zero_t = consts.tile([P, 1], F32, name="zero_t")
            nc.vector.memset(zero_t, 0.0)
            nb_t = consts.tile([P, 1], F32, name="nb_t")
            nc.vector.memset(nb_t, -0.5)
            bias_lhs = consts.tile([P, 2, P], FP8, name="bias_lhs")
            nc.gpsimd.memset(bias_lhs, 0.0)
            nc.gpsimd.memset(bias_lhs[0:1, 0, :], 1.0)
            bv8_b = consts.tile([P, C], FP8, name="bv8_b")
            ap8 = bass.AP(tensor=bvb.tensor, offset=bvb.offset,
                          ap=[[0, P], [1, C]])
            bv_b = bcast16(bvb, C, "bv_b")
            nc.gpsimd.tensor_scalar_mul(out=bv8_b, in0=bv_b, scalar1=WS)
            bap_b = bcast16(bapb, C, "bap_b")
            bmp_b = bcast16(bmpb, C, "bmp_b")
            bq_c = colt(bq, NC, "bq_c")
            bk_c = colt(bk, NC, "bk_c")
            bfc_c = colt(bfc, NF, "bfc_c")

            with tc.tile_pool(name="psA", bufs=3, space="PSUM") as psA, \
                 tc.tile_pool(name="psB", bufs=3, space="PSUM") as psB, \
                 tc.tile_pool(name="psC", bufs=2, space="PSUM") as psC, \
                 tc.tile_pool(name="w8p", bufs=1) as w8p, \
                 tc.tile_pool(name="mtp", bufs=3) as mtp, \
                 tc.tile_pool(name="ptp", bufs=4) as ptp, \
                 tc.tile_pool(name="ysm", bufs=4) as ysmp, \
                 tc.tile_pool(name="xqp", bufs=1) as xqp, \
                 tc.tile_pool(name="lnh", bufs=2) as lnh, \
                 tc.tile_pool(name="lns", bufs=3) as lns, \
                 tc.tile_pool(name="lnx", bufs=2) as lnx:
                for rep in range(repeat):
                    _emit_iteration(upto,
                        nc, tc, d, rep,
                        ident=ident, identDR=identDR, eps_t=eps_t,
                        zero_t=zero_t, nb_t=nb_t, bv8_b=bv8_b,
                        bias_lhs=bias_lhs, bap_b=bap_b,
                        bmp_b=bmp_b, bq_c=bq_c, bk_c=bk_c, bfc_c=bfc_c,
                        xb=xb, xqb=xqb, xq=xq, maskp=maskp, moffs=moffs,
                        wq_p=wq_p, wk_p=wk_p, wv_p=wv_p, wap=wap, wfc=wfc,
                        wmp=wmp, out=out,
                        psA=psA, psB=psB, psC=psC, w8p=w8p, mtp=mtp,
                        ptp=ptp, ysmp=ysmp, xqp=xqp, lnh=lnh, lns=lns,
                        lnx=lnx)
    nc.compile()
    return nc


def _emit_iteration(upto, nc, tc, d, rep, *, ident, identDR, eps_t, zero_t,
                    nb_t, bv8_b, bias_lhs, bap_b, bmp_b, bq_c, bk_c, bfc_c,
                    xb, xqb, xq, maskp, moffs, wq_p, wk_p, wv_p, wap, wfc,
                    wmp, out, psA, psB, psC, w8p, mtp, ptp, ysmp, xqp, lnh, lns, lnx):
    T, C, H, F, HD = d["T"], d["C"], d["H"], d["F"], d["HD"]
    NT, NJ, NC, NF, HPW = d["NT"], d["NJ"], d["NC"], d["NF"], d["HPW"]
    C5, NH5, T5, NT5 = d["C5"], d["NH5"], d["T5"], d["NT5"]
    NHG = d["NHG"]
    TQ = NJ * P
    NP2 = NC // 2                    # DR contraction pairs over C
    sfx = f"_r{rep}"

    # -------- persistent activations (alloc order = reverse free order) -----
    x2sb, free_x2 = tc.tile([P, NJ, C], F32, name="x2sb" + sfx)
    h2T, free_h2T = tc.tile([P, NC, TQ], BF16, name="h2T" + sfx)
    vaug, free_v = tc.tile([P, NT, H, HD + 1], FP8, name="vaug" + sfx)
    qsb, free_q = tc.tile([P, NC, TQ], FP8, name="qsb" + sfx)
    ksb, free_k = tc.tile([P, NC + 1, T], FP8, name="ksb" + sfx)
    ysb, free_y = tc.tile([P, NC, TQ], FP8, name="ysb" + sfx)
    hqT, free_hqT = tc.tile([P, NC, TQ], FP8, name="hqT" + sfx)
    h1T, free_h1T = tc.tile([P, NC, T], FP8, name="h1T" + sfx)
    nc.vector.memset(vaug[:, :, :, HD:HD + 1], WS)
    nc.gpsimd.memset(ksb[:, NC, :], 0.0)    # zeros chunk for score-DR padding

    # -------- resident fp8 weights (few big DMAs, issued by first use) -----
    wqt = w8p.tile([P, NC, NC, P], FP8, name="wqt")
    wkt = w8p.tile([P, NC, NC, P], FP8, name="wkt")
    wvt = w8p.tile([P, NC, C], FP8, name="wvt")
    wapt = w8p.tile([P, NC, C], FP8, name="wapt")

    def emit_w_kv():
        nc.sync.dma_start(out=wkt,
                          in_=wk_p.rearrange("p (mc ci q) -> p mc ci q",
                                             ci=NC, q=P))
        nc.sync.dma_start(out=wvt,
                          in_=wv_p.rearrange("p (ci q) -> p ci q", q=C))

    def emit_w_qp():
        nc.sync.dma_start(out=wqt,
                          in_=wq_p.rearrange("p (mc ci q) -> p mc ci q",
                                             ci=NC, q=P))
        nc.sync.dma_start(out=wapt,
                          in_=wap.rearrange("p (ci q) -> p ci q", q=C))

    # ---------------- LayerNorm helpers ----------------
    def ln_stats(xt):
        """[128, C] -> rstd [P,1], nmu = -mu*rstd (exp(-.5*ln(var+eps)))."""
        ns = max(1, C // 512)
        w = C // ns
        st = lns.tile([P, ns * 6 + 5], F32, name="st")
        stats = st[:, 0:ns * 6].rearrange("p (s x) -> p s x", x=6)
        for s in range(ns):
            nc.vector.bn_stats(out=stats[:, s, :], in_=xt[:, s * w:(s + 1) * w])
        mv = st[:, ns * 6:ns * 6 + 2]
        nc.vector.bn_aggr(out=mv, in_=stats)
        lv = st[:, ns * 6 + 2:ns * 6 + 3]
        nc.scalar.activation(out=lv, in_=mv[:, 1:2], func=AF.Ln,
                             bias=eps_t, scale=1.0)
        rstd = st[:, ns * 6 + 3:ns * 6 + 4]
        nc.scalar.activation(out=rstd, in_=lv, func=AF.Exp,
                             bias=zero_t, scale=-0.5)
        nmu = st[:, ns * 6 + 4:ns * 6 + 5]
        nc.vector.tensor_mul(out=nmu, in0=mv[:, 0:1], in1=rstd)
        nc.vector.tensor_scalar_mul(out=nmu, in0=nmu, scalar1=-1.0)
        return rstd, nmu

    def ln_tile(xt, alt, all_dve=False):
        rstd, nmu = ln_stats(xt)
        hb = lnh.tile([P, C], BF16, name="hb")
        nc.gpsimd.tensor_scalar(out=hb, in0=xt, scalar1=rstd,
                                scalar2=nmu, op0=ALU.mult, op1=ALU.add)
        return hb

    def ln_transposed(hb, dst, idx, use_act=False):
        """transpose [128,C] bf16 into dst chunks (LN gain/bias are folded
        into the consuming weights host-side, so these are plain copies,
        fused 4 chunks per op; early chunks go to the then-idle ACT)."""
        for c0 in range(0, NC, 4):
            pt4 = psB.tile([P, 4, P], BF16, name="sps")
            for i in range(4):
                nc.tensor.transpose(pt4[:, i, :],
                                    hb[:, (c0 + i) * P:(c0 + i + 1) * P], ident)
            if use_act:
                nc.scalar.copy(
                    out=dst[:, c0:c0 + 4, idx * P:(idx + 1) * P], in_=pt4)
            else:
                nc.vector.tensor_copy(
                    out=dst[:, c0:c0 + 4, idx * P:(idx + 1) * P], in_=pt4)

    # ---------------- LN1 helpers (emitted inside the K-chunk loop) -------
    lnp = lnx
    GT = 2                           # token tiles per batched load

    def bq4(wq0):
        b = bq_c[:, wq0:wq0 + 4]
        return bass.AP(tensor=b.tensor, offset=b.offset,
                       ap=[b.ap[0], [1, 4], [0, P]])

    def emit_ln1_pair(g0):
        xts = lnp.tile([P, GT, C], BF16, name="xts")
        nc.sync.dma_start(
            out=xts, in_=xb[g0 * P:(g0 + GT) * P, :]
            .rearrange("(t p) c -> p t c", p=P))
        for tl in range(GT):
            hb = ln_tile(xts[:, tl, :], (g0 + tl) % 2 == 0)
            ln_transposed(hb, h1T, g0 + tl, use_act=(g0 < 8))

    def emit_ln1_xq(j):
        xts = lnp.tile([P, GT, C], BF16, name="xts")
        nc.sync.dma_start(out=xts[:, 0, :], in_=xqb[j * P:(j + 1) * P, :])
        hb = ln_tile(xts[:, 0, :], j % 2 == 0)
        ln_transposed(hb, hqT, j, use_act=(j < 4))

    # ---------------- fused LN1 + K/V/Q + attention + proj + LN2 ----------
    DRN = 256                       # max N per DR matmul (moving = 2*DRN)

    def dr_accum(ps, n0, nn, lhs_of, rhs_of):
        """accumulate over NP2 chunk-pairs into ps[:, n0:n0+nn]."""
        for sub0 in range(0, nn, DRN):
            sw = min(DRN, nn - sub0)
            for cp in range(NP2):
                nc.tensor.matmul(
                    ps[:, n0 + sub0:n0 + sub0 + sw],
                    lhsT=lhs_of(cp), rhs=rhs_of(cp, n0 + sub0, sw),
                    start=(cp == 0), stop=(cp == NP2 - 1), perf_mode=DR)

    def emit_k1(tt, wk):
        ps = psA.tile([P, 512], F32, name="ps")[:, :T5]
        dr_accum(ps, 0, T5,
                 lambda cp: wkt[:, wk, 2 * cp:2 * cp + 2, :],
                 lambda cp, o, w: h1T[:, 2 * cp:2 * cp + 2,
                                      tt * T5 + o:tt * T5 + o + w])
        if tt < 2 or wk % 2 == 0:
            nc.scalar.activation(
                out=ksb[:, wk, tt * T5:(tt + 1) * T5], in_=ps,
                func=AF.Identity, bias=bk_c[:, wk:wk + 1], scale=WSI)
        else:
            nc.vector.tensor_scalar(
                out=ksb[:, wk, tt * T5:(tt + 1) * T5], in0=ps,
                scalar1=WSI, scalar2=bk_c[:, wk:wk + 1],
                op0=ALU.mult, op1=ALU.add)

    hpv = C5 // HD

    def emit_v(t):
        for vh in range(NH5):
            ps = psA.tile([P, 512], F32, name="ps")[:, :C5]
            for sub0 in range(0, C5, DRN):
                sw = min(DRN, C5 - sub0)
                for cp in range(NP2):
                    nc.tensor.matmul(
                        ps[:, sub0:sub0 + sw],
                        lhsT=h1T[:, 2 * cp:2 * cp + 2, t * P:(t + 1) * P],
                        rhs=wvt[:, 2 * cp:2 * cp + 2,
                                vh * C5 + sub0:vh * C5 + sub0 + sw],
                        start=(cp == 0), stop=False, perf_mode=DR)
                nc.tensor.matmul(
                    ps[:, sub0:sub0 + sw], lhsT=bias_lhs,
                    rhs=dup2(bv8_b[:, vh * C5 + sub0:vh * C5 + sub0 + sw], sw),
                    start=False, stop=True, perf_mode=DR)
            dst = vaug[:, t, vh * hpv:(vh + 1) * hpv, 0:HD]
            srcv = ps.rearrange("p (h dd) -> p h dd", dd=HD)
            if t < 8:
                nc.scalar.copy(out=dst, in_=srcv)
            else:
                nc.vector.tensor_copy(out=dst, in_=srcv)

    def emit_q4(j, wq0):
        ps4 = psB.tile([P, 4, P], F32, name="sps")
        for i in range(4):
            dr_accum(ps4[:, i, :], 0, P,
                     lambda cp, wq=wq0 + i: wqt[:, wq, 2 * cp:2 * cp + 2, :],
                     lambda cp, o, w: hqT[:, 2 * cp:2 * cp + 2,
                                          j * P + o:j * P + o + w])
        # bq varies per wq chunk -> per-(partition,chunk) bias tile view
        nc.vector.scalar_tensor_tensor(
            out=qsb[:, wq0:wq0 + 4, j * P:(j + 1) * P], in0=ps4,
            scalar=WSI, in1=bq4(wq0), op0=ALU.mult, op1=ALU.add)

    # ------- attention -------
    _sctr = [0]

    def dup2(apx, n):
        """[p, n] -> [p, 2, n] with stride-0 duplicated middle dim."""
        return bass.AP(tensor=apx.tensor, offset=apx.offset,
                       ap=[apx.ap[0], [0, 2], [1, n]])

    def score_group(j, h, kc0, kw, mj, nkc):
        hp = (h % HPW) * HD
        wk = h // HPW
        hl = h % HG
        _sctr[0] += 1
        pool = psA if _sctr[0] % 2 else psB
        sps = pool.tile([P, 4, P], F32, name="ps" if _sctr[0] % 2 else "sps")
        for ki in range(kw):
            kc = kc0 + ki
            kb = ksb[hp:hp + HD, wk, kc * P:(kc + 1) * P]
            lhsT = bass.AP(tensor=kb.tensor, offset=kb.offset,
                           ap=[kb.ap[0], [(NC - wk) * T, 2], [1, P]])
            qb = qsb[hp:hp + HD, wk, j * P:(j + 1) * P]
            nc.tensor.matmul(sps[:, ki, :], lhsT=lhsT, rhs=dup2(qb, P),
                             start=True, stop=False, perf_mode=DR)
            mb = mj[:, (hl * nkc + kc) * P:(hl * nkc + kc + 1) * P]
            nc.tensor.matmul(sps[:, ki, :], lhsT=identDR, rhs=dup2(mb, P),
                             start=False, stop=True, perf_mode=DR)
        pt = ptp.tile([P, 4, P], FP8, name="pt")
        nc.scalar.activation(out=pt[:, :kw, :], in_=sps[:, :kw, :],
                             func=AF.Exp, bias=nb_t, scale=0.125)
        return pt

    def av_group(h, kc0, kw, pt, yps, nkc, may_start):
        # start=True marks the WHOLE 2KB PSUM bank pending-zero, so only the
        # very first matmul touching this pair's bank may set it; the other
        # head's region is then zeroed by its own first (start=False) write.
        for kp in range(0, kw, 2):
            kc = kc0 + kp
            nc.tensor.matmul(
                yps, lhsT=pt[:, kp:kp + 2, :],
                rhs=vaug[:, kc:kc + 2, h, :],
                start=(may_start and kc == 0), stop=(kc + 2 >= nkc),
                perf_mode=DR)

    def finish_pair(j, hA, yps2):
        wk = hA // 2
        rec = ysmp.tile([P, 2], F32, name="rec")
        nc.vector.reciprocal(out=rec, in_=yps2[:, :, HD])  # [P,2] strided
        ynm = ysmp.tile([P, P], BF16, name="ynm")
        nc.vector.tensor_scalar_mul(out=ynm[:, 0:HD],
                                    in0=yps2[:, 0, 0:HD],
                                    scalar1=rec[:, 0:1])
        nc.vector.tensor_scalar_mul(out=ynm[:, HD:2 * HD],
                                    in0=yps2[:, 1, 0:HD],
                                    scalar1=rec[:, 1:2])
        ypt = psB.tile([P, P], BF16, name="sps")
        nc.tensor.transpose(ypt, ynm, ident)
        nc.vector.tensor_copy(out=ysb[:, wk, j * P:(j + 1) * P], in_=ypt)

    _pend = [None]     # AV lag-1 pipeline state, global across pairs and j

    def flush_av():
        p = _pend[0]
        if p is None:
            return
        j, hA, kc0, kw, ptA, ptB, yps2, nkc, last = p
        av_group(hA, kc0, kw, ptA, yps2[:, 0, 0:HD + 1], nkc, True)
        av_group(hA + 1, kc0, kw, ptB, yps2[:, 1, 0:HD + 1], nkc, False)
        if last:
            finish_pair(j, hA, yps2)
        _pend[0] = None

    bg = []                        # (need_by_j, emit_fn) background quanta

    def bg_pop(need_j):
        i = 0
        while i < len(bg):
            if bg[i][0] <= need_j:
                bg.pop(i)[1]()
            else:
                i += 1

    def bg_pop_one():
        n = 2 if len(bg) > 6 else 1
        for _ in range(min(n, len(bg))):
            bg.pop(0)[1]()

    _mjs = {}

    def emit_mask(j, g):
        nkc = 2 * (j + 1)
        mj = mtp.tile([P, HG * 2 * NJ * P], FP8, name="mj")
        msrc = bass.AP(tensor=maskp.tensor, offset=moffs[(j, g)],
                       ap=[[HG * nkc * P, P], [1, HG * nkc * P]])
        nc.sync.dma_start(out=mj[:, :HG * nkc * P], in_=msrc)
        _mjs.setdefault(j, []).append(mj)

    _mstream = [(j_, g_) for j_ in range(NJ) for g_ in range(NHG)]
    _mptr = [0]

    def mask_ensure(upto):
        while _mptr[0] <= min(upto, len(_mstream) - 1):
            j_, g_ = _mstream[_mptr[0]]
            emit_mask(j_, g_)
            _mptr[0] += 1

    def emit_attention(j):
        nkc = 2 * (j + 1)
        mask_ensure(j * NHG + 1)
        bg_pop(j)                   # anything attention(j) depends on
        mjs = _mjs[j]
        for hA in range(0, H, 2):
            hB = hA + 1
            mask_ensure(j * NHG + hA // HG + 2)
            yps2 = psC.tile([P, 2, 256], F32, name="yps")
            for kc0 in range(0, nkc, 4):
                kw = min(4, nkc - kc0)
                ptA = score_group(j, hA, kc0, kw, mjs[hA // HG], nkc)
                ptB = score_group(j, hB, kc0, kw, mjs[hB // HG], nkc)
                flush_av()
                _pend[0] = (j, hA, kc0, kw, ptA, ptB, yps2, nkc,
                            kc0 + kw >= nkc)
                bg_pop_one()

    def emit_proj_ln2(j):
        xqt = xqp.tile([P, C], F32, name="xqt")
        nc.sync.dma_start(out=xqt, in_=xq[j * P:(j + 1) * P, :])
        pss = []
        for nh in range(NH5):
            ps = psA.tile([P, 512], F32, name="ps")[:, :C5]
            dr_accum(ps, 0, C5,
                     lambda cp: ysb[:, 2 * cp:2 * cp + 2, j * P:(j + 1) * P],
                     lambda cp, o, w: wapt[:, 2 * cp:2 * cp + 2,
                                           nh * C5 + o:nh * C5 + o + w])
            pss.append(ps)
        for nh in range(NH5):
            sl = slice(nh * C5, (nh + 1) * C5)
            nc.vector.scalar_tensor_tensor(
                out=x2sb[:, j, sl], in0=pss[nh], scalar=WSI,
                in1=xqt[:, sl], op0=ALU.mult, op1=ALU.add)
        nc.gpsimd.tensor_add(out=x2sb[:, j, :], in0=x2sb[:, j, :], in1=bap_b)
        hb = ln_tile(x2sb[:, j, :], j % 2 == 0, all_dve=True)
        ln_transposed(hb, h2T, j)

    TPC = T5 // P                  # token tiles per K-chunk

    def bg_for_chunk(tt):
        need = 2 * tt
        q = []
        for g0 in range(tt * TPC, (tt + 1) * TPC, GT):
            q.append((need, (lambda g=g0: emit_ln1_pair(g))))
        for wk in range(0, NC, 2):
            q.append((need, (lambda w=wk: (emit_k1(tt, w),
                                           emit_k1(tt, w + 1)))))
        for t in range(tt * TPC, (tt + 1) * TPC, 2):
            q.append((need, (lambda t_=t: (emit_v(t_), emit_v(t_ + 1)))))
        return q

    def bg_for_j(j):
        q = [(j, lambda: emit_ln1_xq(j))]
        for wq0 in range(0, NC, 4):
            q.append((j, (lambda w=wq0: emit_q4(j, w))))
        return q

    # prologue: chunk 0 + j0 prerequisites, DMAs ordered by first use
    pro = bg_for_chunk(0)
    nln = max(1, TPC // GT)
    for _, f in pro[:nln]:
        f()
    emit_w_kv()
    for _, f in pro[nln:]:
        f()
    emit_w_qp()
    for _, f in bg_for_j(0):
        f()
    if NJ > 1:
        for _, f in bg_for_j(1):
            f()
    for j in range(NJ):
        if j % 2 == 0 and j // 2 + 1 < NT5:
            bg.extend(bg_for_chunk(j // 2 + 1))
        if j + 2 < NJ:
            bg.extend(bg_for_j(j + 2))
        emit_attention(j)
        if j > 0:
            emit_proj_ln2(j - 1)
    bg_pop(NJ)
    flush_av()
    emit_proj_ln2(NJ - 1)
    free_h1T()
    free_hqT()
    free_y()
    free_k()
    free_q()
    free_v()
    if upto <= 2:
        for j in range(NJ):
            nc.sync.dma_start(out=out[j * P:(j + 1) * P, :], in_=x2sb[:, j, :])
        free_h2T(); free_x2()
        return

    # mlp-proj bias folded into the x2 accumulator up front
    for j in range(NJ):
        nc.gpsimd.tensor_add(out=x2sb[:, j, :], in0=x2sb[:, j, :], in1=bmp_b)

    # ---------------- FC+GELU -> aT, then MLP-proj accumulated into x2 -----
    GRP = 4
    NG = NF // GRP
    aT, free_aT = tc.tile([P, NF, TQ], BF16, name="aT" + sfx)
    with tc.tile_pool(name="wfcp" + sfx, bufs=2) as wfcp, \
         tc.tile_pool(name="wmpp" + sfx, bufs=1) as wmpp:
        for g in range(NG):
            wfg = wfcp.tile([P, GRP, NC, P], BF16, name="wfg")
            nc.sync.dma_start(
                out=wfg,
                in_=bass.AP(tensor=wfc.tensor,
                            offset=wfc.offset + g * GRP * NC * P,
                            ap=[[NF * NC * P, P], [1, GRP * NC * P]])
                .rearrange("p (wf ci q) -> p wf ci q", ci=NC, q=P))
            for wfl in range(GRP):
                wf = g * GRP + wfl
                for tq5 in range(TQ // 512 if TQ >= 512 else 1):
                    Q5 = min(512, TQ)
                    ps = psA.tile([P, 512], F32, name="ps")[:, :Q5]
                    for ci in range(NC):
                        nc.tensor.matmul(
                            ps, lhsT=wfg[:, wfl, ci, :],
                            rhs=h2T[:, ci, tq5 * Q5:(tq5 + 1) * Q5],
                            start=(ci == 0), stop=(ci == NC - 1))
                    nc.scalar.activation(
                        out=aT[:, wf, tq5 * Q5:(tq5 + 1) * Q5], in_=ps,
                        func=AF.Gelu_apprx_tanh, bias=bfc_c[:, wf:wf + 1],
                        scale=1.0)
            wmg = wmpp.tile([P, GRP, C], BF16, name="wmg")
            nc.sync.dma_start(
                out=wmg,
                in_=wmp[g * GRP:(g + 1) * GRP, :, :].rearrange(
                    "fi p q -> p fi q"))
            for j in range(NJ):
                pss = []
                for nh in range(NH5):
                    pss.append(psB.tile([P, 512], F32, name="sps")[:, :C5])
                for fi in range(GRP):
                    for nh in range(NH5):
                        nc.tensor.matmul(
                            pss[nh],
                            lhsT=aT[:, g * GRP + fi, j * P:(j + 1) * P],
                            rhs=wmg[:, fi, nh * C5:(nh + 1) * C5],
                            start=(fi == 0), stop=(fi == GRP - 1))
                for nh in range(NH5):
                    sl = slice(nh * C5, (nh + 1) * C5)
                    nc.vector.tensor_add(out=x2sb[:, j, sl],
                                         in0=x2sb[:, j, sl], in1=pss[nh])
    free_aT()
    free_h2T()

    # ---------------- write out ----------------
    for j in range(NJ):
        nc.sync.dma_start(out=out[j * P:(j + 1) * P, :], in_=x2sb[:, j, :])
    free_x2()


# ======================= host side =======================

def prep_shards(inputs, cfg):
    d = cfg_derived(cfg)
    T, C, H, F, HD = d["T"], d["C"], d["H"], d["F"], d["HD"]
    NJ, NC, NF, NHG = d["NJ"], d["NC"], d["NF"], d["NHG"]
    moffs, MTOT = mask_layout(d)

    x = np.ascontiguousarray(np.asarray(inputs["x"], np.float32))
    mask = np.asarray(inputs["fire_causal_mask"], np.float32)[0]  # [H,T,T]
    wqkv = np.asarray(inputs["w_qkv"], np.float32)
    bqkv = np.asarray(inputs["b_qkv"], np.float32)
    # fold LN gains/biases into the consuming weights (device LN emits the
    # pre-gain normalized activations only)
    g1 = np.asarray(inputs["ln1_g"], np.float32)
    b1 = np.asarray(inputs["ln1_b"], np.float32)
    g2 = np.asarray(inputs["ln2_g"], np.float32)
    b2 = np.asarray(inputs["ln2_b"], np.float32)
    wqkv_g = wqkv * g1[:, None]
    bqkv_f = bqkv + b1 @ wqkv
    wfc_raw = np.asarray(inputs["w_fc"], np.float32)
    wfc_g = wfc_raw * g2[:, None]
    bfc_f = np.asarray(inputs["b_fc"], np.float32) + b2 @ wfc_raw

    def q8(w):
        return np.clip(w * WS, -240., 240.).astype(F8NP)

    def tile_kxm_f8(w):
        """[K, M] -> [P, M//P, K//P, P] flat: row p holds w[ci*P+p, mc*P+q]
        packed [mc][ci][q] — the DR lhsT layout, one contiguous DMA."""
        Kd, M = w.shape
        w4 = q8(w).reshape(Kd // P, P, M // P, P)       # [ci, p, mc, q]
        t = w4.transpose(1, 2, 0, 3).reshape(P, -1)     # [p, mc, ci, q]
        return np.ascontiguousarray(t)

    def tile_rhs_f8(w):
        """[K, N] -> [P, K//P * N]: row p holds w[ci*P+p, :] ci-major."""
        Kd, N = w.shape
        t = q8(w).reshape(Kd // P, P, N).transpose(1, 0, 2).reshape(P, -1)
        return np.ascontiguousarray(t)

    def tile_kxm_bf(w):
        Kd, M = w.shape
        w4 = w.astype(BF16NP).reshape(Kd // P, P, M // P, P)
        t = w4.transpose(1, 2, 0, 3).reshape(P, -1)
        return np.ascontiguousarray(t)

    # mask: packed 8*m fp8, per (j, head-group) contiguous [p(kt), h, kc, q]
    m8 = np.clip(mask * 8.0, -240., 240.).astype(F8NP)
    maskps = []
    for par in range(2):
        buf = np.empty(MTOT, F8NP)
        for j in range(NJ):
            nkc = 2 * (j + 1)
            tq = 2 * j + par
            sub = m8[:, tq * P:(tq + 1) * P, :nkc * P]      # [H, q, nkc*P]
            arr = sub.reshape(H, P, nkc, P).transpose(3, 0, 2, 1)
            for g in range(NHG):
                o = moffs[(j, g)]
                blk = arr[:, g * HG:(g + 1) * HG]           # [p, HG, kc, q]
                buf[o:o + blk.size] = blk.ravel()
        maskps.append(buf)

    shared = dict(
        wq_p=tile_kxm_f8(wqkv_g[:, :C]),
        wk_p=tile_kxm_f8(wqkv_g[:, C:2 * C]),
        wv_p=tile_rhs_f8(wqkv_g[:, 2 * C:]),
        wap=tile_rhs_f8(np.asarray(inputs["w_attn_proj"], np.float32)),
        wfc=tile_kxm_bf(wfc_g),
        wmp=np.ascontiguousarray(
            np.asarray(inputs["w_mlp_proj"], np.float32)
            .reshape(NF, P, C).astype(BF16NP)),
        bq=bqkv_f[:C].copy(), bk=bqkv_f[C:2 * C].copy(),
        bvb=bqkv_f[2 * C:].astype(BF16NP),
        bapb=np.asarray(inputs["b_attn_proj"], np.float32).astype(BF16NP),
        bmpb=np.asarray(inputs["b_mlp_proj"], np.float32).astype(BF16NP),
        bfc=bfc_f,
    )
    in_maps = []
    for c in range(8):
        b, par = c // 2, c % 2
        xq_ = np.concatenate(
            [x[b, (2 * j + par) * P:(2 * j + par + 1) * P] for j in range(NJ)], 0)
        m = dict(shared)
        m["xb"] = x[b].astype(BF16NP)
        m["xq"] = np.ascontiguousarray(xq_)
        m["xqb"] = np.ascontiguousarray(xq_).astype(BF16NP)
        m["maskp"] = maskps[par]
        in_maps.append(m)
    return in_maps


def assemble(results, cfg, B=4):
    d = cfg_derived(cfg)
    T, C, NJ = d["T"], d["C"], d["NJ"]
    out = np.zeros((B, T, C), np.float32)
    for c in range(8):
        b, par = c // 2, c % 2
        co = results[c]["out"]
        for j in range(NJ):
            tq = 2 * j + par
            out[b, tq * P:(tq + 1) * P] = co[j * P:(j + 1) * P]
    return out


_GRAPH_CACHE = {}


def kernel(**inputs):
    cfg = FULL
    key = "full"
    if key not in _GRAPH_CACHE:
        _GRAPH_CACHE[key] = build_graph(cfg)
    nc = _GRAPH_CACHE[key]
    in_maps = prep_shards(inputs, cfg)
    res = run_bass_kernel_spmd(nc, in_maps, core_ids=list(range(8)))
    return assemble(res.results, cfg)
